# revision 21
# baseline (speedup 1.0000x reference)
"""GraphUNet (N=4096, E=65536, C=256, depth 3, ratio 0.5) on 8 trn2 NeuronCores.

Row-sharded SPMD pipeline, 6 launches. Device does the O(n^2 C) message
passing (N @ Z SpMMs) and the dense A@A augmentations; host does top-k,
gathers, O(nC^2) feature transforms / weight folds, and O(n^2)
element-wise adjacency prep between launches.

Layout/overlap notes: every DRAM tensor ships partition-major
([128, K/128, F]) so DMA descriptors move >=512B contiguous runs at the
full modeled 360 GB/s; input DMAs are emitted in consumption order with
the streamed operand chunk-interleaved so the PE starts ~4us in; output
DMAs are emitted last (the DMA queue is in-order, so a store's semaphore
wait would block later loads); diag/bias GCN corrections are folded into
the PSUM accumulation as rank-1 / diagonal matmul chains, leaving one
fused tensor_scalar per output block.

  K1   init GCN:   x0[sl] = dis0*(A0[sl] @ (zhi+zlo) + 2*dis0*y0[sl]
                   + (1/dis0) x b), A0 in exact small-int fp8 (DoubleRow),
                   z = dis0*(x@Wi) as an fp8 hi+lo split.
  K2-K4a levels:   M^T col-block = R^T @ L[sl]^T chain in fp8 (exact ints;
                   bf16 at level 3), shipped back raw (bf16-exact ints);
                   the down GCN reuses the in-SBUF M^T block as lhsT
                   against host-prescaled yp = dis*(xp@Wd), with the
                   diag/bias corrections in-chain and relu fused into the
                   dis row-scale consumer.
  K4b  up GCNs:    xU1 = relu(N2 @ z2 + b) replicated (z2 host-folds the
                   deepest unpool scatter), then xU2[sl] = relu((N1[sl]@xd1
                   + N1[sl][:,perm1]@xU1) @ Wu1 + b) with one on-device
                   transpose; biases enter the psum via rank-1 ones chains.
  K4c  final GCN:  identical program to K1 on zf = (x0 + scatter(perm0,
                   xU2)) @ Wf -- the unpool scatter and Wf fold on host
                   collapse the two chains of the reference into one.

Precision: adjacency chains exact; features bf16 (fp8 hi+lo where they
feed the big A-chains); measured end-to-end rel err ~7e-3 vs f32 ref.
"""

import numpy as np
import ml_dtypes

from contextlib import ExitStack

import concourse.bass as bass
import concourse.mybir as mybir
import concourse.tile as tile
from concourse import bacc
from concourse.bass_utils import run_bass_kernel_spmd

NCORES = 8
C = 256
F32 = mybir.dt.float32
F32R = mybir.dt.float32r
BF16 = mybir.dt.bfloat16
FP8 = mybir.dt.float8e4

NP_OF = {F32: np.float32, F32R: np.float32,
         BF16: ml_dtypes.bfloat16, FP8: ml_dtypes.float8_e4m3fn}

_TRACE = {"on": False, "results": [], "ncs": []}
DR = mybir.MatmulPerfMode.DoubleRow
MULT = mybir.AluOpType.mult
MAXOP = mybir.AluOpType.max


def _new_nc():
    return bacc.Bacc("TRN2", target_bir_lowering=False, debug=False,
                     num_devices=NCORES)


def _finish(nc):
    nc.compile()
    _TRACE["ncs"].append(nc)
    return nc


def _run(nc, in_maps):
    res = run_bass_kernel_spmd(nc, in_maps, list(range(NCORES)),
                               trace=_TRACE["on"])
    if _TRACE["on"]:
        _TRACE["results"].append(res)
    return res.results


def _ld(nc, t, dram, k0, k1, f0=None, f1=None):
    if f0 is None:
        nc.sync.dma_start(t[:, k0:k1, :], dram.ap()[:, k0:k1, :])
    else:
        nc.sync.dma_start(t[:, k0:k1, f0:f1], dram.ap()[:, k0:k1, f0:f1])


# ------------------------------------------------------------ K1 / K4c
def build_diag():
    """x[sl] = dis[sl]*((A0+2I)[sl] @ (zhi+zlo)) + (1/dis)xb scaled.
    [4096 -> 512/core].  The +2I diagonal folds into the fp8 adjacency
    (still exact small ints).

    AT ships mo-major so each 128-row output chain completes as soon as
    its A-slice lands; consumers and stores pipeline behind the PE."""
    n, rpc, KT, MO = 4096, 512, 32, 4
    nc = _new_nc()
    AT = nc.dram_tensor("AT", [128, MO, KT, 128], FP8,
                        kind="ExternalInput")
    ZH = nc.dram_tensor("ZH", [128, KT, C], FP8, kind="ExternalInput")
    ZL = nc.dram_tensor("ZL", [128, KT, C], FP8, kind="ExternalInput")
    DISP = nc.dram_tensor("DISP", [128, MO], F32, kind="ExternalInput")
    IV = nc.dram_tensor("IV", [1, rpc], BF16, kind="ExternalInput")
    BV = nc.dram_tensor("BV", [1, C], BF16, kind="ExternalInput")
    XO = nc.dram_tensor("XO", [128, MO, C], F32, kind="ExternalOutput")
    with tile.TileContext(nc) as tc:
        ctx = ExitStack()
        sb = ctx.enter_context(tc.tile_pool(name="sb", bufs=1))
        ps = ctx.enter_context(tc.tile_pool(name="ps", bufs=1, space="PSUM"))
        at_sb = sb.tile([128, MO, KT, 128], FP8, tag="at")
        zh_sb = sb.tile([128, KT, C], FP8, tag="zh")
        zl_sb = sb.tile([128, KT, C], FP8, tag="zl")
        disp_sb = sb.tile([128, MO], F32, tag="disp")
        iv_sb = sb.tile([128, rpc], BF16, tag="iv")
        bv_sb = sb.tile([128, C], BF16, tag="bv")
        nc.sync.dma_start(at_sb[:, 0, :, :], AT.ap()[:, 0, :, :])
        nc.sync.dma_start(disp_sb[:], DISP.ap())
        nc.sync.dma_start(iv_sb[:1, :], IV.ap())
        nc.sync.dma_start(bv_sb[:1, :], BV.ap())
        for k0 in range(0, KT, 16):
            _ld(nc, zh_sb, ZH, k0, k0 + 16)
            _ld(nc, zl_sb, ZL, k0, k0 + 16)
        nc.sync.dma_start(at_sb[:, 1, :, :], AT.ap()[:, 1, :, :])
        nc.sync.dma_start(at_sb[:, 2, :, :], AT.ap()[:, 2, :, :])
        nc.sync.dma_start(at_sb[:, 3, 0:16, :], AT.ap()[:, 3, 0:16, :])
        nc.sync.dma_start(at_sb[:, 3, 16:32, :], AT.ap()[:, 3, 16:32, :])
        xo_sb = sb.tile([128, MO, C], F32, tag="xo")
        for mo in range(MO):
            pso = ps.tile([128, C], F32, tag=f"p{mo}", name=f"p{mo}")
            for kp in range(KT // 2):
                for ci, ch in enumerate((zh_sb, zl_sb)):
                    nc.tensor.matmul(
                        pso[:], at_sb[:, mo, 2 * kp:2 * kp + 2, :],
                        ch[:, 2 * kp:2 * kp + 2, :],
                        start=(kp == 0 and ci == 0), stop=False,
                        perf_mode=DR)
            nc.tensor.matmul(pso[:],
                             iv_sb[:1, mo * 128:(mo + 1) * 128],
                             bv_sb[:1, :], start=False, stop=True)
            nc.any.tensor_scalar_mul(xo_sb[:, mo, :], pso[:],
                                     disp_sb[:, mo:mo + 1])
        for mo in range(MO):
            nc.sync.dma_start(XO.ap()[:, mo, :], xo_sb[:, mo, :])
        ctx.close()
    return _finish(nc)


# ----------------------------------------------------- K2 / K3 / K4a
def build_level(npv, n, rpc, adt, ship):
    """M^T col-block = R^T @ L[sl]^T (exact ints), then
    x[sl] = relu(dis[sl] * (M[sl] @ yp + diag(2-diagM) yp[sl]
    + (1/dis) x b)).  Ships raw M^T if `ship`."""
    KTp, KT = npv // 128, n // 128
    mo2 = (rpc + 127) // 128
    mdt = BF16 if ship else F32R
    ydt = BF16 if ship else F32R
    nc = _new_nc()
    R = nc.dram_tensor("R", [128, KTp, n], adt, kind="ExternalInput")
    LT = nc.dram_tensor("LT", [128, KTp, rpc], adt, kind="ExternalInput")
    YP = nc.dram_tensor("YP", [128, KT, C], ydt, kind="ExternalInput")
    DISP = nc.dram_tensor("DISP", [128, mo2], F32, kind="ExternalInput")
    DG = nc.dram_tensor("DG", [128, mo2, 128], ydt, kind="ExternalInput")
    YS = nc.dram_tensor("YS", [128, mo2, C], ydt, kind="ExternalInput")
    IV = nc.dram_tensor("IV", [1, max(rpc, 128)], ydt,
                        kind="ExternalInput")
    BV = nc.dram_tensor("BV", [1, C], ydt, kind="ExternalInput")
    if ship:
        MT = nc.dram_tensor("MT", [128, KT, rpc], BF16,
                            kind="ExternalOutput")
    XO = nc.dram_tensor("XO", [128, mo2, C] if rpc >= 128 else [rpc, C],
                        F32, kind="ExternalOutput")
    groups = [list(range(g, min(g + 6, KT))) for g in range(0, KT, 6)]
    with tile.TileContext(nc) as tc:
        ctx = ExitStack()
        sb = ctx.enter_context(tc.tile_pool(name="sb", bufs=1))
        ps = ctx.enter_context(tc.tile_pool(name="ps", bufs=1, space="PSUM"))
        disp_sb = sb.tile([128, mo2], F32, tag="disp")
        dg_sb = sb.tile([128, mo2, 128], ydt, tag="dg")
        ys_sb = sb.tile([128, mo2, C], ydt, tag="ys")
        iv_sb = sb.tile([128, max(rpc, 128)], ydt, tag="iv")
        bv_sb = sb.tile([128, C], ydt, tag="bv")
        lt_sb = sb.tile([128, KTp, rpc], adt, tag="lt")
        for k0 in range(0, KTp, 8):
            _ld(nc, lt_sb, LT, k0, min(KTp, k0 + 8))
        r_sb = sb.tile([128, KTp, n], adt, tag="r")
        yp_sb = sb.tile([128, KT, C], ydt, tag="yp")
        for gi, mos in enumerate(groups):
            f0, f1 = mos[0] * 128, (mos[-1] + 1) * 128
            for k0 in range(0, KTp, 8):
                _ld(nc, r_sb, R, k0, min(KTp, k0 + 8), f0, f1)
            if gi == 0:
                nc.sync.dma_start(disp_sb[:], DISP.ap())
                nc.sync.dma_start(dg_sb[:], DG.ap())
                nc.sync.dma_start(ys_sb[:], YS.ap())
                nc.sync.dma_start(iv_sb[:1, :], IV.ap())
                nc.sync.dma_start(bv_sb[:1, :], BV.ap())
                for k0 in range(0, KT, 8):
                    _ld(nc, yp_sb, YP, k0, min(KT, k0 + 8))
        mt_sb = sb.tile([128, KT, rpc], mdt, tag="mt")
        xo_sb = sb.tile([128, mo2, C], F32, tag="xo")
        use_dr = adt == FP8
        gps = [ps.tile([128, C], F32, tag=f"g{m}", name=f"g{m}")
               for m in range(mo2)]
        msz2 = min(128, rpc)
        for gi, mos in enumerate(groups):
            pss = {m: ps.tile([128, rpc], F32, tag=f"p{m % 6}",
                              name=f"p{m}") for m in mos}
            if use_dr:
                for kp in range(KTp // 2):
                    for mo in mos:
                        nc.tensor.matmul(
                            pss[mo][:],
                            r_sb[:, 2 * kp:2 * kp + 2,
                                 mo * 128:(mo + 1) * 128],
                            lt_sb[:, 2 * kp:2 * kp + 2, :],
                            start=(kp == 0), stop=(kp == KTp // 2 - 1),
                            perf_mode=DR)
            else:
                for kt in range(KTp):
                    for mo in mos:
                        nc.tensor.matmul(
                            pss[mo][:],
                            r_sb[:, kt, mo * 128:(mo + 1) * 128],
                            lt_sb[:, kt, :],
                            start=(kt == 0), stop=(kt == KTp - 1))
            for mo in mos:
                nc.any.tensor_copy(mt_sb[:, mo, :], pss[mo][:])
            for m in range(mo2):
                for kt in mos:
                    nc.tensor.matmul(
                        gps[m][:msz2, :],
                        mt_sb[:, kt, m * 128:m * 128 + msz2],
                        yp_sb[:, kt, :], start=(kt == 0), stop=False)
        for m in range(mo2):
            nc.tensor.matmul(gps[m][:msz2, :], dg_sb[:msz2, m, :msz2],
                             ys_sb[:msz2, m, :], start=False, stop=False)
            nc.tensor.matmul(gps[m][:msz2, :],
                             iv_sb[:1, m * 128:m * 128 + msz2],
                             bv_sb[:1, :], start=False, stop=True)
            nc.vector.tensor_scalar(xo_sb[:msz2, m, :], gps[m][:msz2, :],
                                    disp_sb[:msz2, m:m + 1], 0.0,
                                    MULT, MAXOP)
        if ship:
            for gi, mos in enumerate(groups):
                nc.sync.dma_start(MT.ap()[:, mos[0]:mos[-1] + 1, :],
                                  mt_sb[:, mos[0]:mos[-1] + 1, :])
        if rpc >= 128:
            for m in range(mo2):
                nc.sync.dma_start(XO.ap()[:, m, :], xo_sb[:, m, :])
        else:
            nc.sync.dma_start(XO.ap(), xo_sb[:rpc, 0, :])
        ctx.close()
    return _finish(nc)


# ------------------------------------------------------------------ K4b
def build_k4b():
    """xU1 = relu(N2 @ z2 + b0) replicated;
    xU2[sl] = relu((N1[sl]@xd1 + N1[sl][:,perm1]@xU1) @ Wu1 + b1)."""
    nc = _new_nc()
    NT2 = nc.dram_tensor("NT2", [128, 8, 1024], BF16, kind="ExternalInput")
    Z2 = nc.dram_tensor("Z2", [128, 8, C], BF16, kind="ExternalInput")
    NT1B = nc.dram_tensor("NT1B", [128, 16, C], BF16, kind="ExternalInput")
    XD1 = nc.dram_tensor("XD1", [128, 16, C], BF16, kind="ExternalInput")
    Q1B = nc.dram_tensor("Q1B", [128, 8, C], BF16, kind="ExternalInput")
    WU1 = nc.dram_tensor("WU1", [128, 2, C], BF16, kind="ExternalInput")
    IDT = nc.dram_tensor("IDT", [128, 128], BF16, kind="ExternalInput")
    ONES = nc.dram_tensor("ONES", [1, 128], BF16, kind="ExternalInput")
    B0 = nc.dram_tensor("B0", [1, C], BF16, kind="ExternalInput")
    B1 = nc.dram_tensor("B1", [1, C], BF16, kind="ExternalInput")
    XO = nc.dram_tensor("XO", [128, 2, C], F32, kind="ExternalOutput")
    with tile.TileContext(nc) as tc:
        ctx = ExitStack()
        sb = ctx.enter_context(tc.tile_pool(name="sb", bufs=1))
        ps = ctx.enter_context(tc.tile_pool(name="ps", bufs=1, space="PSUM"))
        z2_sb = sb.tile([128, 8, C], BF16, tag="z2")
        nc.sync.dma_start(z2_sb[:], Z2.ap())
        nt2_sb = sb.tile([128, 8, 1024], BF16, tag="nt2")
        for k0 in range(0, 8, 2):
            _ld(nc, nt2_sb, NT2, k0, k0 + 2)
        ones_sb = sb.tile([128, 128], BF16, tag="ones")
        nc.sync.dma_start(ones_sb[:1, :], ONES.ap())
        b0_sb = sb.tile([128, C], BF16, tag="b0")
        nc.sync.dma_start(b0_sb[:1, :], B0.ap())
        b1_sb = sb.tile([128, C], BF16, tag="b1")
        nc.sync.dma_start(b1_sb[:1, :], B1.ap())
        idt_sb = sb.tile([128, 128], BF16, tag="idt")
        nc.sync.dma_start(idt_sb[:], IDT.ap())
        wu1_sb = sb.tile([128, 2, C], BF16, tag="wu1")
        nc.sync.dma_start(wu1_sb[:], WU1.ap())
        xd1_sb = sb.tile([128, 16, C], BF16, tag="xd1")
        for k0 in range(0, 16, 8):
            _ld(nc, xd1_sb, XD1, k0, k0 + 8)
        nt1b_sb = sb.tile([128, 16, C], BF16, tag="nt1b")
        for k0 in range(0, 16, 8):
            _ld(nc, nt1b_sb, NT1B, k0, k0 + 8)
        q1b_sb = sb.tile([128, 8, C], BF16, tag="q1b")
        nc.sync.dma_start(q1b_sb[:], Q1B.ap())

        xu1_sb = sb.tile([128, 8, C], BF16, tag="xu1")
        # v2 psums get dedicated banks so the xd1 part can accumulate while
        # xU1 is still being produced
        pv = [ps.tile([128, C], F32, tag=f"v{m}", name=f"v{m}")
              for m in range(2)]
        xu1_groups = [list(range(6)), [6, 7]]
        for mos in xu1_groups:
            pss = {m: ps.tile([128, C], F32, tag=f"p{m % 6}",
                              name=f"pu{m}") for m in mos}
            for kt in range(8):
                for mo in mos:
                    nc.tensor.matmul(
                        pss[mo][:], nt2_sb[:, kt, mo * 128:(mo + 1) * 128],
                        z2_sb[:, kt, :], start=(kt == 0), stop=False)
            for mo in mos:
                nc.tensor.matmul(pss[mo][:], ones_sb[:1, :], b0_sb[:1, :],
                                 start=False, stop=True)
                nc.vector.tensor_scalar_max(xu1_sb[:, mo, :], pss[mo][:],
                                            0.0)
            if mos[0] == 0:
                # xd1 part of v2 can start as soon as its inputs land
                for mo in range(2):
                    for kt in range(16):
                        nc.tensor.matmul(
                            pv[mo][:],
                            nt1b_sb[:, kt, mo * 128:(mo + 1) * 128],
                            xd1_sb[:, kt, :], start=(kt == 0), stop=False)
        v2_sb = sb.tile([128, 2, C], BF16, tag="v2")
        for mo in range(2):
            for kt in range(8):
                nc.tensor.matmul(
                    pv[mo][:], q1b_sb[:, kt, mo * 128:(mo + 1) * 128],
                    xu1_sb[:, kt, :], start=False, stop=(kt == 7))
            nc.any.tensor_copy(v2_sb[:, mo, :], pv[mo][:])
        v2t_sb = sb.tile([128, 2, C], BF16, tag="v2t")
        for mo in range(2):
            for cc in range(2):
                pst = ps.tile([128, 128], BF16, tag=f"p{2 + mo * 2 + cc}",
                              name="pt")
                nc.tensor.transpose(pst[:],
                                    v2_sb[:, mo, cc * 128:(cc + 1) * 128],
                                    idt_sb[:])
                nc.any.tensor_copy(v2t_sb[:, cc, mo * 128:(mo + 1) * 128],
                                   pst[:])
        xo_sb = sb.tile([128, 2, C], F32, tag="xo")
        for mo in range(2):
            ps3 = ps.tile([128, C], F32, tag=f"v{mo}", name="pw")
            for kt in range(2):
                nc.tensor.matmul(
                    ps3[:], v2t_sb[:, kt, mo * 128:(mo + 1) * 128],
                    wu1_sb[:, kt, :], start=(kt == 0), stop=False)
            nc.tensor.matmul(ps3[:], ones_sb[:1, :], b1_sb[:1, :],
                             start=False, stop=True)
            nc.vector.tensor_scalar_max(xo_sb[:, mo, :], ps3[:], 0.0)
        nc.sync.dma_start(XO.ap(), xo_sb[:])
        ctx.close()
    return _finish(nc)


# =================================================================== host
F8NP = ml_dtypes.float8_e4m3fn
BFNP = ml_dtypes.bfloat16


def _pm(a, dt):
    """[K, F] row-major -> partition-major [128, K//128, F]."""
    K, F = a.shape
    return np.ascontiguousarray(
        a.reshape(K // 128, 128, F).transpose(1, 0, 2)).astype(dt)


def _unpm(b):
    """[128, KT, F] -> [K, F]."""
    p, kt, f = b.shape
    return np.asarray(b, np.float32).transpose(1, 0, 2).reshape(kt * p, f)


def _pmv(v):
    """[K] -> [128, K//128] partition-major (padded to 128 rows)."""
    k = v.shape[0]
    if k < 128:
        v = np.pad(v, (0, 128 - k))
        k = 128
    return np.ascontiguousarray(
        v.reshape(k // 128, 128).T).astype(np.float32)


def _dgblk(c, dt):
    """[rpc] diag values -> [128, mo2, 128] block-diagonal lhsT."""
    rpc = c.shape[0]
    mo2 = (rpc + 127) // 128
    out = np.zeros((128, mo2, 128), np.float32)
    for m in range(mo2):
        seg = c[m * 128:(m + 1) * 128]
        out[np.arange(len(seg)), m, np.arange(len(seg))] = seg
    return out.astype(dt)


def _rowvec(v, width, dt):
    out = np.zeros((1, width), np.float32)
    out[0, :v.shape[0]] = v
    return out.astype(dt)


def _mk_dis(deg):
    return (1.0 / np.sqrt(np.maximum(deg, 1e-12))).astype(np.float32)


def _diag_inputs(A8T, z, dis, bvec, rpc):
    """Per-core in_maps for the K1/K4c program. A8T is [4096, 4096] fp8
    (= (A0+2I)^T); the per-core AT block ships mo-major
    [128, MO, KT, 128]."""
    zhi = z.astype(F8NP)
    zlo = (z - zhi.astype(np.float32)).astype(F8NP)
    zhi_pm, zlo_pm = _pm(zhi, F8NP), _pm(zlo, F8NP)
    bv = np.asarray(bvec, np.float32)[None, :].astype(BFNP)
    n = A8T.shape[0]
    maps = []
    for c in range(NCORES):
        sl = slice(c * rpc, (c + 1) * rpc)
        blk = A8T[:, sl]                       # [n, rpc]
        at = np.ascontiguousarray(
            blk.reshape(n // 128, 128, rpc // 128, 128)
            .transpose(1, 2, 0, 3))            # [128, MO, KT, 128]
        maps.append({
            "AT": at,
            "ZH": zhi_pm, "ZL": zlo_pm,
            "DISP": _pmv(dis[sl]),
            "IV": _rowvec(1.0 / dis[sl], rpc, BFNP),
            "BV": bv,
            })
    return maps


def kernel(x, edge_index, W_init, b_init, W_down, b_down, p_pool,
           W_up, b_up, W_final, b_final):
    x = np.asarray(x, np.float32)
    N = x.shape[0]
    rpc0 = N // NCORES

    A0 = np.zeros((N, N), np.float32)
    np.add.at(A0, (np.asarray(edge_index[0]), np.asarray(edge_index[1])),
              1.0)
    dis0 = _mk_dis(A0.sum(1) + 2.0)
    y0 = x @ np.asarray(W_init, np.float32)

    # exact level-0 score via host matvec (init GCN is linear)
    p0 = np.asarray(p_pool[0], np.float32)
    u = y0 @ p0
    s0 = (dis0 * (A0 @ (dis0 * u)) + 2.0 * dis0 * dis0 * u) \
        / np.linalg.norm(p0)
    perm0 = np.argsort(-s0, kind="stable")[:N // 2]
    sv0 = s0[perm0]

    # ---- K1
    A2I = np.ascontiguousarray(A0.T)               # (A0+2I)^T
    A2I[np.arange(N), np.arange(N)] += 2.0
    A8T = A2I.astype(F8NP)                         # [4096, 4096]
    nc1 = build_diag()
    maps = _diag_inputs(A8T, dis0[:, None] * y0, dis0,
                        np.asarray(b_init, np.float32), rpc0)
    res = _run(nc1, maps)
    x0 = np.concatenate([_unpm(r["XO"]) for r in res], 0)

    # ---- down levels
    Bh = A0 + np.eye(N, dtype=np.float32)
    xcur, perm, sv = x0, perm0, sv0
    n = N
    Ms, dis_l, xs, perms = [], [dis0], [x0], []
    level_fp8 = [True, True, False]
    for lev in range(3):
        k = n // 2
        rpc = k // NCORES
        perms.append(perm)
        L = Bh[perm, :]
        R = Bh[:, perm]
        lim = 16 if level_fp8[lev] else 256
        assert Bh.max() <= lim, (lev, Bh.max())
        diagM = np.einsum('it,ti->i', L, R, optimize=True)
        deg = L @ R.sum(1) - diagM + 2.0
        dis = _mk_dis(deg)
        xp = xcur[perm] * np.tanh(sv)[:, None]
        y = xp @ np.asarray(W_down[lev], np.float32)
        adt = FP8 if level_fp8[lev] else BF16
        npdt = NP_OF[adt]
        ship = lev < 2
        ydt = BFNP if ship else np.float32
        nc = build_level(n, k, rpc, adt, ship)
        Rpm = _pm(R, npdt)
        yfull = (dis[:, None] * y).astype(np.float32)
        yp_pm = _pm(yfull, ydt)
        bvec = np.asarray(b_down[lev], np.float32)
        maps = []
        for cc in range(NCORES):
            sl = slice(cc * rpc, (cc + 1) * rpc)
            maps.append({
                "R": Rpm,
                "LT": _pm(np.ascontiguousarray(L[sl].T), npdt),
                "YP": yp_pm,
                "DISP": _pmv(dis[sl]),
                "DG": _dgblk(2.0 - diagM[sl], ydt),
                "YS": _pm(yfull[sl], ydt) if rpc >= 128 else
                np.ascontiguousarray(
                    np.pad(yfull[sl], ((0, 128 - rpc), (0, 0)))
                    [:, None, :]).astype(ydt),
                "IV": _rowvec(1.0 / dis[sl], max(rpc, 128), ydt),
                "BV": bvec[None, :].astype(ydt),
                })
        res = _run(nc, maps)
        if rpc >= 128:
            xn = np.concatenate([_unpm(r["XO"]) for r in res], 0)
        else:
            xn = np.concatenate([np.asarray(r["XO"], np.float32)
                                 for r in res], 0)
        if ship:
            M = np.concatenate([_unpm(r["MT"]).T for r in res], 0)
            Ms.append(M)
            Bh = M.copy()
            np.fill_diagonal(Bh, 1.0)
        dis_l.append(dis)
        xs.append(xn)
        xcur, n = xn, k
        if lev < 2:
            pl = np.asarray(p_pool[lev + 1], np.float32)
            s = xn @ pl / np.linalg.norm(pl)
            perm = np.argsort(-s, kind="stable")[:k // 2]
            sv = s[perm]

    x_d1, x_d2, x_d3 = xs[1], xs[2], xs[3]
    dis1, dis2 = dis_l[1], dis_l[2]
    M1, M2 = Ms[0], Ms[1]
    perm1, perm2 = perms[1], perms[2]

    # ---- K4b
    N2 = M2.copy()
    np.fill_diagonal(N2, 2.0)
    N2 *= dis2[:, None] * dis2[None, :]
    N1 = M1.copy()
    np.fill_diagonal(N1, 2.0)
    N1 *= dis1[:, None] * dis1[None, :]
    up = np.zeros_like(x_d2)
    up[perm2] = x_d3
    z2 = (x_d2 + up) @ np.asarray(W_up[0], np.float32)
    nc4b = build_k4b()
    rpc1 = 2048 // NCORES
    nt2_pm = _pm(np.ascontiguousarray(N2.T), BFNP)
    z2_pm = _pm(z2, BFNP)
    xd1_pm = _pm(x_d1, BFNP)
    wu1_pm = _pm(np.asarray(W_up[1], np.float32), BFNP)
    idt = np.eye(128, dtype=np.float32).astype(BFNP)
    ones = np.ones((1, 128), BFNP)
    b0 = np.asarray(b_up[0], np.float32)[None, :].astype(BFNP)
    b1 = np.asarray(b_up[1], np.float32)[None, :].astype(BFNP)
    maps = []
    for cc in range(NCORES):
        sl = slice(cc * rpc1, (cc + 1) * rpc1)
        maps.append({
            "NT2": nt2_pm, "Z2": z2_pm,
            "NT1B": _pm(np.ascontiguousarray(N1[sl].T), BFNP),
            "XD1": xd1_pm,
            "Q1B": _pm(np.ascontiguousarray(N1[sl][:, perm1].T), BFNP),
            "WU1": wu1_pm, "IDT": idt, "ONES": ones, "B0": b0, "B1": b1,
            })
    res = _run(nc4b, maps)
    xU2 = np.concatenate([_unpm(r["XO"]) for r in res], 0)

    # ---- K4c
    upf = np.zeros_like(x0)
    upf[perm0] = xU2
    zf = (x0 + upf) @ np.asarray(W_final, np.float32)
    nc4c = build_diag()
    maps = _diag_inputs(A8T, dis0[:, None] * zf, dis0,
                        np.asarray(b_final, np.float32), rpc0)
    res = _run(nc4c, maps)
    out = np.concatenate([_unpm(r["XO"]) for r in res], 0)
    return out.astype(np.float32)
